# revision 58
# baseline (speedup 1.0000x reference)
"""Trainium2 Bass kernel for multi-head GQA attention (B=2, S=2048, D=2048,
H=16 query heads, 4 KV head groups), distributed over 8 NeuronCores.

Sharding: core c handles batch b = c//4 and KV-head-group g = c%4 (query heads
4g..4g+3).  W_q/W_k/W_v column-parallel per group; attention computed fully
locally per group; W_o ROW-parallel: each core multiplies its local attention
output [S, 512] by its W_o row-slice [512, 2048] producing a full-width
partial, which is ReduceScattered (bf16, add) within each batch's 4-core
replica group into the final [S, 512] column slice.

The kernel runs as a per-chunk pipeline (causal): for each 512-row i-chunk,
project K/V/Q for that chunk, run attention against all previous K/V chunks,
apply W_o, and kick the chunk's two half-ReduceScatters.  This staggers the
collective chain from ~70us onward so it drains during compute instead of
piling into a tail (the CC core processes collectives serially at ~20us per
1MB half).

All matmuls run in bf16 with fp32 PSUM accumulation.  Softmax skips
max-subtraction (scores are bounded for these inputs).  The softmax
denominator is built by summing the transposed-P tiles elementwise on the
Vector engine (bf16) as they are produced, then one ones-matmul broadcasts
the partition-sum, reciprocal_approx_fast inverts it, and the normalization
is applied on the attn@V PSUM copy-out.
"""

import math

import ml_dtypes
import numpy as np

import concourse.bass as bass
import concourse.mybir as mybir
import concourse.tile as tile
from concourse import bacc
from concourse.bass_utils import run_bass_kernel_spmd
from concourse.masks import make_identity

BF16 = np.dtype(ml_dtypes.bfloat16)
N_CORES = 8
B, S, D = 2, 2048, 2048
H, G = 16, 4            # query heads, group size
HKV = H // G            # 4 kv heads == 4 groups
HD = D // H             # 128
P = 128                 # partitions
CH = 512                # i/j chunk width
NCH = S // CH           # 4 chunks
KT = D // P             # 16 k-tiles for the projections
NH = H // HKV           # 4 local query heads per core
NJT = S // P            # 16 j-tiles
SCALE = 1.0 / math.sqrt(HD)

_DT = mybir.dt.bfloat16
_F32 = mybir.dt.float32


def _build(mode: str):
    """mode: 'causal' (tril mask), 'full' (no mask), 'addmask' (generic
    additive mask input [S, S])."""
    nc = bacc.Bacc("TRN2", target_bir_lowering=False, debug=False,
                   num_devices=N_CORES)

    # pre-tiled host layouts: per-partition-contiguous for fat DMA descriptors
    xq = nc.dram_tensor("xq", [NCH, P, KT * CH], _DT, kind="ExternalInput").ap()
    xk = nc.dram_tensor("xk", [NCH, P, KT * CH], _DT, kind="ExternalInput").ap()
    xv = nc.dram_tensor("xv", [NCH, P, KT * CH], _DT, kind="ExternalInput").ap()
    wq = nc.dram_tensor("wq", [P, KT * NH * HD], _DT, kind="ExternalInput").ap()
    wk = nc.dram_tensor("wk", [P, KT * HD], _DT, kind="ExternalInput").ap()
    wv = nc.dram_tensor("wv", [P, KT * HD], _DT, kind="ExternalInput").ap()
    wo = nc.dram_tensor("wo", [P, NH * D], _DT, kind="ExternalInput").ap()
    cs = nc.dram_tensor("cs", [P, S], _DT, kind="ExternalInput").ap()
    if mode == "causal":
        cmask = nc.dram_tensor("cmask", [P, P], _DT, kind="ExternalInput").ap()
    elif mode == "addmask":
        amask = nc.dram_tensor("amask", [S, S], _DT, kind="ExternalInput").ap()
    out = nc.dram_tensor("out", [S, CH], _DT, kind="ExternalOutput").ap()

    def nch_of(ic):
        return (ic + 1) if mode == "causal" else NCH

    with tile.TileContext(nc) as tc:
        cpool = tc.alloc_tile_pool(name="const", bufs=1)
        ones_mat = cpool.tile([P, P], _DT)
        nc.gpsimd.memset(ones_mat[:], 1.0)
        if mode == "causal":
            cmask_sb = cpool.tile([P, P], _DT)
            nc.sync.dma_start(cmask_sb[:], cmask[:])

        # resident K^T / V (attention reads all previous chunks)
        rpool = tc.alloc_tile_pool(name="resident", bufs=1)
        kpt_sb = rpool.tile([P, S], _DT)              # roped K^T [hd, S]
        vp_sb = rpool.tile([P, NJT, HD], _DT)         # V [j-tile, d] per tile

        from contextlib import ExitStack
        with ExitStack() as stack:
            pool = lambda *a, **kw: stack.enter_context(tc.tile_pool(*a, **kw))
            xpool = pool(name="proj", bufs=4)
            wpool = pool(name="projw", bufs=1)
            tpool = pool(name="ropet", bufs=3)
            qpool = pool(name="qp", bufs=2)
            apool = pool(name="ap", bufs=2)
            ptpool = pool(name="pt", bufs=2)
            accpool = pool(name="accp", bufs=2)
            bcpool = pool(name="bcp", bufs=2)
            popool = pool(name="pop", bufs=12)
            spool = pool(name="small", bufs=8)
            wowpool = pool(name="wow", bufs=1)
            dpool = pool(name="dram", bufs=4, space="DRAM")
            drpool = pool(name="dramr", bufs=8, space="DRAM")
            mm_ps = pool(name="mm_ps", bufs=3, space="PSUM")
            sc_ps = pool(name="sc_ps", bufs=3, space="PSUM")
            av_ps = pool(name="av_ps", bufs=2, space="PSUM")

            def load_x(src, ic, pieces=1):
                x_sb = xpool.tile([P, KT, CH], _DT, tag="x", name="x")
                step = KT // pieces
                for tp in range(pieces):
                    nc.sync.dma_start(
                        x_sb[:, tp * step:(tp + 1) * step, :].rearrange(
                            "p a b -> p (a b)"),
                        src[ic][:, tp * step * CH:(tp + 1) * step * CH])
                return x_sb

            def rope(dst, psum, ic):
                # stage PSUM->SBUF via the scalar engine so the 6 DVE ops run
                # on bf16 SBUF operands instead of f32 PSUM
                pc = tpool.tile([P, CH], _DT, tag="ropeC", name="ropeC")
                nc.scalar.activation(out=pc[:], in_=psum[:],
                                     func=mybir.ActivationFunctionType.Copy)
                c = cs_sb[0:64, ic * CH:(ic + 1) * CH]       # base 0
                s = cs_sb[64:128, ic * CH:(ic + 1) * CH]     # base 64
                s0 = cs2_sb[0:64, ic * CH:(ic + 1) * CH]     # sin at base 0
                c64 = cs2_sb[64:128, ic * CH:(ic + 1) * CH]  # cos at base 64
                re = pc[0:64, :]
                im = pc[64:128, :]
                t1 = tpool.tile([64, CH], _DT, tag="ropeA", name="ropeA")
                t2 = tpool.tile([64, CH], _DT, tag="ropeB", name="ropeB")
                lo = dst[0:64, :]
                hi = dst[64:128, :]
                nc.vector.tensor_tensor(out=t1[:], in0=re, in1=c, op=mybir.AluOpType.mult)
                nc.vector.tensor_tensor(out=t2[:], in0=im, in1=s, op=mybir.AluOpType.mult)
                nc.vector.tensor_sub(out=lo, in0=t1[:], in1=t2[:])
                nc.vector.tensor_tensor(out=t1[:], in0=re, in1=s0, op=mybir.AluOpType.mult)
                nc.vector.tensor_tensor(out=t2[:], in0=im, in1=c64, op=mybir.AluOpType.mult)
                nc.vector.tensor_add(out=hi, in0=t1[:], in1=t2[:])

            # initial loads: wk first (first matmul), then chunk-0 x tensors
            wk_sb = wpool.tile([P, KT, HD], _DT)
            nc.sync.dma_start(wk_sb[:].rearrange("p a b -> p (a b)"), wk[:])
            xk_t = {0: load_x(xk, 0, pieces=4)}
            cs_sb = wpool.tile([P, S], _DT)
            nc.sync.dma_start(cs_sb[:], cs[:])
            # swapped-half copy [s; c] so rope's cross products pair equal
            # SBUF base partitions (SB-SB tensor_tensor constraint)
            cs2_sb = wpool.tile([P, S], _DT)
            nc.sync.dma_start(cs2_sb[0:64, :], cs[64:128, :])
            nc.sync.dma_start(cs2_sb[64:128, :], cs[0:64, :])
            wv_sb = wpool.tile([P, KT, HD], _DT)
            nc.sync.dma_start(wv_sb[:].rearrange("p a b -> p (a b)"), wv[:])
            xv_t = {0: load_x(xv, 0, pieces=2)}
            xq_t = {0: load_x(xq, 0, pieces=2)}
            wq_sb = wpool.tile([P, KT, NH * HD], _DT)
            nc.sync.dma_start(wq_sb[:].rearrange("p a b -> p (a b)"), wq[:])
            wo_sb = wowpool.tile([P, NH, D], _DT)
            nc.sync.dma_start(wo_sb[:].rearrange("p a b -> p (a b)"), wo[:])

            def proj_kv(kc):
                # K projection + rope into kpt_sb
                x_sb = xk_t.pop(kc)
                ps = mm_ps.tile([P, CH], _F32, tag="mm", name="pjk")
                for t in range(KT):
                    nc.tensor.matmul(ps[:], lhsT=wk_sb[:, t, :], rhs=x_sb[:, t, :],
                                     start=(t == 0), stop=(t == KT - 1))
                rope(kpt_sb[:, kc * CH:(kc + 1) * CH], ps, kc)
                # V projection straight into [j, d] layout: x as the stationary
                # operand (lhsT) gives out[j, d] with no transpose step; the 4
                # j-tiles accumulate into disjoint quarters of one PSUM bank
                x_sb = xv_t.pop(kc)
                vps = mm_ps.tile([P, CH], _F32, tag="mm", name="pjv")
                for jb in range(4):
                    for t in range(KT):
                        nc.tensor.matmul(vps[:, jb * HD:(jb + 1) * HD],
                                         lhsT=x_sb[:, t, jb * P:(jb + 1) * P],
                                         rhs=wv_sb[:, t, :],
                                         start=(t == 0), stop=(t == KT - 1),
                                         skip_group_check=True)
                nc.scalar.activation(
                    out=vp_sb[:, 4 * kc:4 * (kc + 1), :].rearrange("p t d -> p (t d)"),
                    in_=vps[:], func=mybir.ActivationFunctionType.Copy)

            rs_outs = {}
            last_bounce = None
            for ic in range(NCH):
                if mode == "causal":
                    proj_kv(ic)
                    # prefetch next chunk's inputs
                    if ic + 1 < NCH:
                        xk_t[ic + 1] = load_x(xk, ic + 1, pieces=2)
                        xv_t[ic + 1] = load_x(xv, ic + 1, pieces=2)
                        xq_t[ic + 1] = load_x(xq, ic + 1, pieces=2)
                    if ic == 0:
                        # tiny warmup collective: absorbs the first-call CC
                        # plan-staging latency and the proj-phase core skew
                        # while the CC is otherwise idle (the first real RS
                        # measured 2-3x the steady-state duration without it)
                        wu_in = dpool.tile([4 * 64, 64], _DT, tag="wui",
                                           name="wui")
                        wu_out = dpool.tile([64, 64], _DT, tag="wuo",
                                            name="wuo")
                        nc.gpsimd.collective_compute(
                            "ReduceScatter", mybir.AluOpType.add,
                            replica_groups=[[0, 1, 2, 3], [4, 5, 6, 7]],
                            ins=[wu_in[:].opt()], outs=[wu_out[:].opt()])
                else:
                    if ic == 0:
                        proj_kv(0)
                        for kc in range(1, NCH):
                            xk_t[kc] = load_x(xk, kc, pieces=2)
                            xv_t[kc] = load_x(xv, kc, pieces=2)
                            proj_kv(kc)
                    if ic + 1 < NCH:
                        xq_t[ic + 1] = load_x(xq, ic + 1, pieces=2)

                # Q projection + rope for this chunk
                x_sb = xq_t.pop(ic)
                qpt = []
                for h in range(NH):
                    ps = mm_ps.tile([P, CH], _F32, tag="mm", name="pjq")
                    for t in range(KT):
                        nc.tensor.matmul(
                            ps[:], lhsT=wq_sb[:, t, h * HD:(h + 1) * HD],
                            rhs=x_sb[:, t, :], start=(t == 0), stop=(t == KT - 1))
                    qh = qpool.tile([P, CH], _DT, tag=f"qpt{h}", name=f"qpt{h}")
                    rope(qh, ps, ic)
                    qpt.append(qh)

                njt = 4 * nch_of(ic)
                at_t = []
                with nc.named_scope(f"attn{ic}"):
                    for h in range(NH):
                        # scores computed TRANSPOSED: sT[j, i] via K-stationary
                        # matmuls; exp writes P^T tiles (no memset: the masked
                        # [0:off) region is never read downstream)
                        pt = ptpool.tile([P, NJT, CH], _DT, tag="pt", name="pt")
                        acc = accpool.tile([P, CH], _DT, tag="acc", name="acc")
                        offs = []
                        for jt in range(njt):
                            jrel = jt - 4 * ic if mode == "causal" else -1
                            off = jrel * P if jrel > 0 else 0
                            w = CH - off
                            offs.append(off)
                            ps = sc_ps.tile([P, CH], _F32, tag="sc", name="sc")
                            nc.tensor.matmul(
                                ps[:, 0:w], lhsT=kpt_sb[:, jt * P:(jt + 1) * P],
                                rhs=qpt[h][:, off:CH],
                                start=True, stop=True)
                            if mode == "causal" and jrel >= 0:
                                # in-block triangle on the (jt == i-tile) block
                                nc.vector.tensor_tensor(
                                    out=ps[:, 0:P], in0=ps[:, 0:P],
                                    in1=cmask_sb[:], op=mybir.AluOpType.add)
                            elif mode == "addmask":
                                am = spool.tile([P, CH], _DT, tag="am", name="am")
                                nc.sync.dma_start(
                                    am[:], amask[jt * P:(jt + 1) * P,
                                                 ic * CH:(ic + 1) * CH])
                                nc.vector.tensor_tensor(
                                    out=ps[:], in0=ps[:], in1=am[:],
                                    op=mybir.AluOpType.add)
                            nc.scalar.activation(
                                out=pt[:, jt, off:CH], in_=ps[:, 0:w],
                                func=mybir.ActivationFunctionType.Exp, scale=SCALE)
                            # denominator pre-sum (bf16, width-restricted),
                            # interleaved with the scores/exp pipeline
                            if jt == 1:
                                o1 = offs[1]
                                nc.vector.tensor_add(
                                    out=acc[:, o1:], in0=pt[:, 0, o1:],
                                    in1=pt[:, 1, o1:])
                                if o1 > 0:
                                    nc.vector.tensor_copy(
                                        out=acc[:, 0:o1], in_=pt[:, 0, 0:o1])
                            elif jt > 1:
                                nc.vector.tensor_add(
                                    out=acc[:, off:], in0=acc[:, off:],
                                    in1=pt[:, jt, off:])

                        # attn @ V -> outT [d, i-chunk] (before dn so the PE
                        # never stalls on the DVE pre-sum chain)
                        ops = av_ps.tile([P, CH], _F32, tag="av", name="av")
                        for jt in range(njt):
                            off = offs[jt]
                            nc.tensor.matmul(ops[:, off:], lhsT=vp_sb[:, jt, :],
                                             rhs=pt[:, jt, off:],
                                             start=(jt == 0), stop=(jt == njt - 1))
                        # denominator: broadcast partition-sum, fast reciprocal
                        # (shares the av pool's two banks: av/dn alternate)
                        dps = av_ps.tile([P, CH], _F32, tag="av", name="dn")
                        nc.tensor.matmul(dps[:], lhsT=ones_mat[:], rhs=acc[:],
                                         start=True, stop=True)
                        bc_sb = bcpool.tile([P, CH], _F32, tag="bcs", name="bcs")
                        nc.vector.reciprocal_approx_fast(out=bc_sb[:], in_=dps[:])
                        ah = apool.tile([P, CH], _DT, tag=f"at{h}", name=f"at{h}")
                        nc.vector.tensor_tensor(
                            out=ah[:], in0=ops[:], in1=bc_sb[:],
                            op=mybir.AluOpType.mult)
                        at_t.append(ah)

                # W_o row-parallel: partial[i, 0:2048] from local heads only.
                # Collectives are issued only AFTER every po DMA of the chunk
                # (shared DMA-completion lanes: a collective in the middle of
                # the po stream makes later po waits wait on the whole RS).
                # collective split: one whole-chunk RS while the chain has
                # compute to hide behind (latency floor ~20us per collective),
                # four quarter-RS for the last chunk to shrink the exposed tail
                nsp = 4 if ic == NCH - 1 else 1
                spw = 4 // nsp
                with nc.named_scope(f"wo{ic}"):
                    bounces = [dpool.tile([4, spw, P, CH], _DT,
                                          tag=f"bounce{ic}_{sp}",
                                          name=f"bounce{sp}")
                               for sp in range(nsp)]
                    def emit_rs(sp):
                        # ReduceScatter(add): rank g of the batch group
                        # receives sum of partial[:, g*512:(g+1)*512]
                        rs_out = drpool.tile([spw * P, CH], _DT,
                                             tag=f"rso{spw}", name="rso")
                        nc.gpsimd.collective_compute(
                            "ReduceScatter", mybir.AluOpType.add,
                            replica_groups=[[0, 1, 2, 3], [4, 5, 6, 7]],
                            ins=[bounces[sp][:].opt()],
                            outs=[rs_out[:].opt()])
                        rs_outs.setdefault(ic, []).append((ic * 4 + sp * spw,
                                                           spw, rs_out))

                    for tl in range(4):
                        for o in range(4):
                            ps = mm_ps.tile([P, CH], _F32, tag="mm", name="wops")
                            for dt_ in range(NH):
                                nc.tensor.matmul(
                                    ps[:], lhsT=at_t[dt_][:, tl * P:(tl + 1) * P],
                                    rhs=wo_sb[:, dt_, o * CH:(o + 1) * CH],
                                    start=(dt_ == 0), stop=(dt_ == NH - 1))
                            # copy on DVE: the scalar engine's exp stream is at
                            # ~parity with the PE and must not be HOL-blocked
                            po = popool.tile([P, CH], _DT, tag="po", name="po")
                            nc.vector.tensor_copy(out=po[:], in_=ps[:])
                            last_bounce = nc.sync.dma_start(
                                bounces[tl // spw][o, tl % spw], po[:])
                        if nsp == 4:
                            # last chunk: fire each quarter as its data lands
                            # (nothing queues behind it, so the mid-stream
                            # lane-coupling hazard doesn't apply here)
                            emit_rs(tl)
                    if nsp != 4:
                        for sp in range(nsp):
                            emit_rs(sp)
                # previous chunk's out-copies, pinned behind this chunk's last
                # bounce write: the tile scheduler otherwise hoists them right
                # behind their ReduceScatter, where the RS peer-wait head-of-
                # line blocks the issuing queue for the next chunk's work
                if ic > 0:
                    for tl0, spw_, rs_out in rs_outs.pop(ic - 1):
                        cp = nc.sync.dma_start(
                            out[tl0 * P:(tl0 + spw_) * P, :], rs_out[:])
                        tile.add_dep_helper(
                            cp.ins, last_bounce.ins, sync=False,
                            reason="out-copy after next chunk's bounces")
            for tl0, spw_, rs_out in rs_outs.pop(NCH - 1):
                cp = nc.sync.dma_start(out[tl0 * P:(tl0 + spw_) * P, :], rs_out[:])
                tile.add_dep_helper(cp.ins, last_bounce.ins, sync=False,
                                    reason="tail out-copy after last bounces")
        rpool.release()
        cpool.release()

    nc.compile()
    return nc


_CACHE = {}


def _get_nc(mode):
    if mode not in _CACHE:
        _CACHE[mode] = _build(mode)
    return _CACHE[mode]


def _tile_x(xt):
    """[D, S] -> [NCH, P, KT*CH] with [ic][p][t*CH+f] = xt[t*P+p][ic*CH+f]."""
    return np.ascontiguousarray(
        xt.reshape(KT, P, NCH, CH).transpose(2, 1, 0, 3).reshape(NCH, P, KT * CH))


def _tile_w(w):
    """[D, N] -> [P, KT*N] with [p][t*N+n] = w[t*P+p][n]."""
    n = w.shape[1]
    return np.ascontiguousarray(
        w.reshape(KT, P, n).transpose(1, 0, 2).reshape(P, KT * n))


def _tile_wo_rows(w):
    """[512, D] -> [P, NH*D] with [p][h*D+o] = w[h*128+p][o]."""
    return np.ascontiguousarray(
        w.reshape(NH, P, D).transpose(1, 0, 2).reshape(P, NH * D))


def _host_prep(q, k, v, mask, freq_cos, freq_sin, W_q, W_k, W_v, W_o):
    q = np.asarray(q, np.float32)
    k = np.asarray(k, np.float32)
    v = np.asarray(v, np.float32)
    W_q = np.asarray(W_q, np.float32)
    W_k = np.asarray(W_k, np.float32)
    W_v = np.asarray(W_v, np.float32)
    W_o = np.asarray(W_o, np.float32)
    cos = np.asarray(freq_cos, np.float32)
    sin = np.asarray(freq_sin, np.float32)
    mask = np.asarray(mask)

    tril = np.tril(np.ones((S, S), np.int32))
    if all(np.array_equal(mask[b], tril) for b in range(B)):
        mode = "causal"
    elif (mask == 1).all():
        mode = "full"
    else:
        mode = "addmask"

    # rope de-interleave permutation for head-dim pairing
    perm = np.concatenate([np.arange(0, HD, 2), np.arange(1, HD, 2)])
    cs = np.concatenate([cos.T, sin.T], axis=0).astype(BF16)   # [128, S]

    if mode == "causal":
        # transposed-scores diagonal block: sT[jj, ii] allowed iff jj <= ii
        jj = np.arange(P)[:, None]
        ii = np.arange(P)[None, :]
        cmask = np.where(jj <= ii, 0.0, -1e9).astype(np.float32).astype(BF16)

    in_maps = []
    for c in range(N_CORES):
        b, g = divmod(c, 4)
        wq_g = W_q[:, g * 512:(g + 1) * 512].copy()
        for l in range(NH):
            wq_g[:, l * HD:(l + 1) * HD] = wq_g[:, l * HD + perm]
        wk_g = W_k[:, g * HD:(g + 1) * HD][:, perm]
        wv_g = W_v[:, g * HD:(g + 1) * HD]
        wo_g = W_o[g * 512:(g + 1) * 512, :]
        m = {
            "xq": _tile_x(q[b].T.astype(BF16)),
            "xk": _tile_x(k[b].T.astype(BF16)),
            "xv": _tile_x(v[b].T.astype(BF16)),
            "wq": _tile_w(wq_g.astype(BF16)),
            "wk": _tile_w(wk_g.astype(BF16)),
            "wv": _tile_w(wv_g.astype(BF16)),
            "wo": _tile_wo_rows(wo_g.astype(BF16)),
            "cs": cs,
        }
        if mode == "causal":
            m["cmask"] = cmask
        elif mode == "addmask":
            # transposed orientation: amask[j, i]
            m["amask"] = np.ascontiguousarray(
                (mask[b].astype(np.float32).T - 1.0) * 1e9).astype(BF16)
        in_maps.append(m)
    return mode, in_maps


def kernel(q, k, v, mask, freq_cos, freq_sin, W_q, W_k, W_v, W_o,
           heads=16, group_size=4, _trace=False, _trace_kwargs=None):
    assert int(heads) == H and int(group_size) == G
    mode, in_maps = _host_prep(q, k, v, mask, freq_cos, freq_sin,
                               W_q, W_k, W_v, W_o)
    nc = _get_nc(mode)
    kw = {}
    if _trace:
        kw = dict(trace=True, **(_trace_kwargs or {}))
    res = run_bass_kernel_spmd(nc, in_maps, core_ids=list(range(N_CORES)), **kw)
    out = np.empty((B, S, D), np.float32)
    for c in range(N_CORES):
        b, g = divmod(c, 4)
        out[b, :, g * 512:(g + 1) * 512] = np.asarray(
            res.results[c]["out"]).astype(np.float32)
    if _trace:
        kernel._last_result = res
    return out


# revision 59
# speedup vs baseline: 1.0129x; 1.0129x over previous
"""Trainium2 Bass kernel for multi-head GQA attention (B=2, S=2048, D=2048,
H=16 query heads, 4 KV head groups), distributed over 8 NeuronCores.

Sharding: core c handles batch b = c//4 and KV-head-group g = c%4 (query heads
4g..4g+3).  W_q/W_k/W_v column-parallel per group; attention computed fully
locally per group; W_o ROW-parallel: each core multiplies its local attention
output [S, 512] by its W_o row-slice [512, 2048] producing a full-width
partial, which is ReduceScattered (bf16, add) within each batch's 4-core
replica group into the final [S, 512] column slice.

The kernel runs as a per-chunk pipeline (causal): for each 512-row i-chunk,
project K/V/Q for that chunk, run attention against all previous K/V chunks,
apply W_o, and kick the chunk's two half-ReduceScatters.  This staggers the
collective chain from ~70us onward so it drains during compute instead of
piling into a tail (the CC core processes collectives serially at ~20us per
1MB half).

All matmuls run in bf16 with fp32 PSUM accumulation.  Softmax skips
max-subtraction (scores are bounded for these inputs).  The softmax
denominator is built by summing the transposed-P tiles elementwise on the
Vector engine (bf16) as they are produced, then one ones-matmul broadcasts
the partition-sum, reciprocal_approx_fast inverts it, and the normalization
is applied on the attn@V PSUM copy-out.
"""

import math

import ml_dtypes
import numpy as np

import concourse.bass as bass
import concourse.mybir as mybir
import concourse.tile as tile
from concourse import bacc
from concourse.bass_utils import run_bass_kernel_spmd
from concourse.masks import make_identity

BF16 = np.dtype(ml_dtypes.bfloat16)
N_CORES = 8
B, S, D = 2, 2048, 2048
H, G = 16, 4            # query heads, group size
HKV = H // G            # 4 kv heads == 4 groups
HD = D // H             # 128
P = 128                 # partitions
CH = 512                # i/j chunk width
NCH = S // CH           # 4 chunks
KT = D // P             # 16 k-tiles for the projections
NH = H // HKV           # 4 local query heads per core
NJT = S // P            # 16 j-tiles
SCALE = 1.0 / math.sqrt(HD)

_DT = mybir.dt.bfloat16
_F32 = mybir.dt.float32


def _build(mode: str):
    """mode: 'causal' (tril mask), 'full' (no mask), 'addmask' (generic
    additive mask input [S, S])."""
    nc = bacc.Bacc("TRN2", target_bir_lowering=False, debug=False,
                   num_devices=N_CORES)

    # pre-tiled host layouts: per-partition-contiguous for fat DMA descriptors
    xq = nc.dram_tensor("xq", [NCH, P, KT * CH], _DT, kind="ExternalInput").ap()
    xk = nc.dram_tensor("xk", [NCH, P, KT * CH], _DT, kind="ExternalInput").ap()
    xv = nc.dram_tensor("xv", [NCH, P, KT * CH], _DT, kind="ExternalInput").ap()
    wq = nc.dram_tensor("wq", [P, KT * NH * HD], _DT, kind="ExternalInput").ap()
    wk = nc.dram_tensor("wk", [P, KT * HD], _DT, kind="ExternalInput").ap()
    wv = nc.dram_tensor("wv", [P, KT * HD], _DT, kind="ExternalInput").ap()
    wo = nc.dram_tensor("wo", [P, NH * D], _DT, kind="ExternalInput").ap()
    cs = nc.dram_tensor("cs", [P, S], _DT, kind="ExternalInput").ap()
    if mode == "causal":
        cmask = nc.dram_tensor("cmask", [P, P], _DT, kind="ExternalInput").ap()
    elif mode == "addmask":
        amask = nc.dram_tensor("amask", [S, S], _DT, kind="ExternalInput").ap()
    out = nc.dram_tensor("out", [S, CH], _DT, kind="ExternalOutput").ap()

    def nch_of(ic):
        return (ic + 1) if mode == "causal" else NCH

    with tile.TileContext(nc) as tc:
        cpool = tc.alloc_tile_pool(name="const", bufs=1)
        ones_mat = cpool.tile([P, P], _DT)
        nc.gpsimd.memset(ones_mat[:], 1.0)
        if mode == "causal":
            cmask_sb = cpool.tile([P, P], _DT)
            nc.sync.dma_start(cmask_sb[:], cmask[:])

        # resident K^T / V (attention reads all previous chunks)
        rpool = tc.alloc_tile_pool(name="resident", bufs=1)
        kpt_sb = rpool.tile([P, S], _DT)              # roped K^T [hd, S]
        vp_sb = rpool.tile([P, NJT, HD], _DT)         # V [j-tile, d] per tile

        from contextlib import ExitStack
        with ExitStack() as stack:
            pool = lambda *a, **kw: stack.enter_context(tc.tile_pool(*a, **kw))
            xpool = pool(name="proj", bufs=4)
            wpool = pool(name="projw", bufs=1)
            tpool = pool(name="ropet", bufs=3)
            qpool = pool(name="qp", bufs=2)
            apool = pool(name="ap", bufs=2)
            ptpool = pool(name="pt", bufs=2)
            accpool = pool(name="accp", bufs=2)
            bcpool = pool(name="bcp", bufs=2)
            popool = pool(name="pop", bufs=12)
            spool = pool(name="small", bufs=8)
            wowpool = pool(name="wow", bufs=1)
            dpool = pool(name="dram", bufs=4, space="DRAM")
            drpool = pool(name="dramr", bufs=8, space="DRAM")
            mm_ps = pool(name="mm_ps", bufs=3, space="PSUM")
            sc_ps = pool(name="sc_ps", bufs=3, space="PSUM")
            av_ps = pool(name="av_ps", bufs=2, space="PSUM")

            def load_x(src, ic, pieces=1):
                x_sb = xpool.tile([P, KT, CH], _DT, tag="x", name="x")
                step = KT // pieces
                for tp in range(pieces):
                    nc.sync.dma_start(
                        x_sb[:, tp * step:(tp + 1) * step, :].rearrange(
                            "p a b -> p (a b)"),
                        src[ic][:, tp * step * CH:(tp + 1) * step * CH])
                return x_sb

            def rope(dst, psum, ic):
                # stage PSUM->SBUF via the scalar engine so the 6 DVE ops run
                # on bf16 SBUF operands instead of f32 PSUM
                pc = tpool.tile([P, CH], _DT, tag="ropeC", name="ropeC")
                nc.scalar.activation(out=pc[:], in_=psum[:],
                                     func=mybir.ActivationFunctionType.Copy)
                c = cs_sb[0:64, ic * CH:(ic + 1) * CH]       # base 0
                s = cs_sb[64:128, ic * CH:(ic + 1) * CH]     # base 64
                s0 = cs2_sb[0:64, ic * CH:(ic + 1) * CH]     # sin at base 0
                c64 = cs2_sb[64:128, ic * CH:(ic + 1) * CH]  # cos at base 64
                re = pc[0:64, :]
                im = pc[64:128, :]
                t1 = tpool.tile([64, CH], _DT, tag="ropeA", name="ropeA")
                t2 = tpool.tile([64, CH], _DT, tag="ropeB", name="ropeB")
                lo = dst[0:64, :]
                hi = dst[64:128, :]
                nc.vector.tensor_tensor(out=t1[:], in0=re, in1=c, op=mybir.AluOpType.mult)
                nc.vector.tensor_tensor(out=t2[:], in0=im, in1=s, op=mybir.AluOpType.mult)
                nc.vector.tensor_sub(out=lo, in0=t1[:], in1=t2[:])
                nc.vector.tensor_tensor(out=t1[:], in0=re, in1=s0, op=mybir.AluOpType.mult)
                nc.vector.tensor_tensor(out=t2[:], in0=im, in1=c64, op=mybir.AluOpType.mult)
                nc.vector.tensor_add(out=hi, in0=t1[:], in1=t2[:])

            # initial loads: wk first (first matmul), then chunk-0 x tensors
            wk_sb = wpool.tile([P, KT, HD], _DT)
            nc.sync.dma_start(wk_sb[:].rearrange("p a b -> p (a b)"), wk[:])
            xk_t = {0: load_x(xk, 0, pieces=4)}
            cs_sb = wpool.tile([P, S], _DT)
            nc.sync.dma_start(cs_sb[:], cs[:])
            # swapped-half copy [s; c] so rope's cross products pair equal
            # SBUF base partitions (SB-SB tensor_tensor constraint)
            cs2_sb = wpool.tile([P, S], _DT)
            nc.sync.dma_start(cs2_sb[0:64, :], cs[64:128, :])
            nc.sync.dma_start(cs2_sb[64:128, :], cs[0:64, :])
            wv_sb = wpool.tile([P, KT, HD], _DT)
            nc.sync.dma_start(wv_sb[:].rearrange("p a b -> p (a b)"), wv[:])
            xv_t = {0: load_x(xv, 0, pieces=2)}
            xq_t = {0: load_x(xq, 0, pieces=2)}
            wq_sb = wpool.tile([P, KT, NH * HD], _DT)
            nc.sync.dma_start(wq_sb[:].rearrange("p a b -> p (a b)"), wq[:])
            wo_sb = wowpool.tile([P, NH, D], _DT)
            nc.sync.dma_start(wo_sb[:].rearrange("p a b -> p (a b)"), wo[:])

            def proj_kv(kc):
                # K projection + rope into kpt_sb
                x_sb = xk_t.pop(kc)
                ps = mm_ps.tile([P, CH], _F32, tag="mm", name="pjk")
                for t in range(KT):
                    nc.tensor.matmul(ps[:], lhsT=wk_sb[:, t, :], rhs=x_sb[:, t, :],
                                     start=(t == 0), stop=(t == KT - 1))
                rope(kpt_sb[:, kc * CH:(kc + 1) * CH], ps, kc)
                # V projection straight into [j, d] layout: x as the stationary
                # operand (lhsT) gives out[j, d] with no transpose step; the 4
                # j-tiles accumulate into disjoint quarters of one PSUM bank
                x_sb = xv_t.pop(kc)
                vps = mm_ps.tile([P, CH], _F32, tag="mm", name="pjv")
                for jb in range(4):
                    for t in range(KT):
                        nc.tensor.matmul(vps[:, jb * HD:(jb + 1) * HD],
                                         lhsT=x_sb[:, t, jb * P:(jb + 1) * P],
                                         rhs=wv_sb[:, t, :],
                                         start=(t == 0), stop=(t == KT - 1),
                                         skip_group_check=True)
                nc.scalar.activation(
                    out=vp_sb[:, 4 * kc:4 * (kc + 1), :].rearrange("p t d -> p (t d)"),
                    in_=vps[:], func=mybir.ActivationFunctionType.Copy)

            rs_outs = {}
            last_bounce = None
            for ic in range(NCH):
                if mode == "causal":
                    proj_kv(ic)
                    # prefetch next chunk's inputs
                    if ic + 1 < NCH:
                        xk_t[ic + 1] = load_x(xk, ic + 1, pieces=2)
                        xv_t[ic + 1] = load_x(xv, ic + 1, pieces=2)
                        xq_t[ic + 1] = load_x(xq, ic + 1, pieces=2)
                    if ic == 0:
                        # tiny warmup collective: absorbs the first-call CC
                        # plan-staging latency and the proj-phase core skew
                        # while the CC is otherwise idle (the first real RS
                        # measured 2-3x the steady-state duration without it)
                        wu_in = dpool.tile([4 * 64, 64], _DT, tag="wui",
                                           name="wui")
                        wu_out = dpool.tile([64, 64], _DT, tag="wuo",
                                            name="wuo")
                        nc.gpsimd.collective_compute(
                            "ReduceScatter", mybir.AluOpType.add,
                            replica_groups=[[0, 1, 2, 3], [4, 5, 6, 7]],
                            ins=[wu_in[:].opt()], outs=[wu_out[:].opt()])
                else:
                    if ic == 0:
                        proj_kv(0)
                        for kc in range(1, NCH):
                            xk_t[kc] = load_x(xk, kc, pieces=2)
                            xv_t[kc] = load_x(xv, kc, pieces=2)
                            proj_kv(kc)
                    if ic + 1 < NCH:
                        xq_t[ic + 1] = load_x(xq, ic + 1, pieces=2)

                # Q projection + rope for this chunk
                x_sb = xq_t.pop(ic)
                qpt = []
                for h in range(NH):
                    ps = mm_ps.tile([P, CH], _F32, tag="mm", name="pjq")
                    for t in range(KT):
                        nc.tensor.matmul(
                            ps[:], lhsT=wq_sb[:, t, h * HD:(h + 1) * HD],
                            rhs=x_sb[:, t, :], start=(t == 0), stop=(t == KT - 1))
                    qh = qpool.tile([P, CH], _DT, tag=f"qpt{h}", name=f"qpt{h}")
                    rope(qh, ps, ic)
                    qpt.append(qh)

                njt = 4 * nch_of(ic)
                at_t = []
                with nc.named_scope(f"attn{ic}"):
                    for h in range(NH):
                        # scores computed TRANSPOSED: sT[j, i] via K-stationary
                        # matmuls; exp writes P^T tiles (no memset: the masked
                        # [0:off) region is never read downstream)
                        pt = ptpool.tile([P, NJT, CH], _DT, tag="pt", name="pt")
                        acc = accpool.tile([P, CH], _DT, tag="acc", name="acc")
                        offs = []
                        for jt in range(njt):
                            jrel = jt - 4 * ic if mode == "causal" else -1
                            off = jrel * P if jrel > 0 else 0
                            w = CH - off
                            offs.append(off)
                            ps = sc_ps.tile([P, CH], _F32, tag="sc", name="sc")
                            nc.tensor.matmul(
                                ps[:, 0:w], lhsT=kpt_sb[:, jt * P:(jt + 1) * P],
                                rhs=qpt[h][:, off:CH],
                                start=True, stop=True)
                            if mode == "causal" and jrel >= 0:
                                # in-block triangle on the (jt == i-tile) block
                                nc.vector.tensor_tensor(
                                    out=ps[:, 0:P], in0=ps[:, 0:P],
                                    in1=cmask_sb[:], op=mybir.AluOpType.add)
                            elif mode == "addmask":
                                am = spool.tile([P, CH], _DT, tag="am", name="am")
                                nc.sync.dma_start(
                                    am[:], amask[jt * P:(jt + 1) * P,
                                                 ic * CH:(ic + 1) * CH])
                                nc.vector.tensor_tensor(
                                    out=ps[:], in0=ps[:], in1=am[:],
                                    op=mybir.AluOpType.add)
                            nc.scalar.activation(
                                out=pt[:, jt, off:CH], in_=ps[:, 0:w],
                                func=mybir.ActivationFunctionType.Exp, scale=SCALE)
                            # denominator pre-sum (bf16, width-restricted),
                            # interleaved with the scores/exp pipeline
                            if jt == 1:
                                o1 = offs[1]
                                nc.vector.tensor_add(
                                    out=acc[:, o1:], in0=pt[:, 0, o1:],
                                    in1=pt[:, 1, o1:])
                                if o1 > 0:
                                    nc.vector.tensor_copy(
                                        out=acc[:, 0:o1], in_=pt[:, 0, 0:o1])
                            elif jt > 1:
                                nc.vector.tensor_add(
                                    out=acc[:, off:], in0=acc[:, off:],
                                    in1=pt[:, jt, off:])

                        # attn @ V -> outT [d, i-chunk] (before dn so the PE
                        # never stalls on the DVE pre-sum chain)
                        ops = av_ps.tile([P, CH], _F32, tag="av", name="av")
                        for jt in range(njt):
                            off = offs[jt]
                            nc.tensor.matmul(ops[:, off:], lhsT=vp_sb[:, jt, :],
                                             rhs=pt[:, jt, off:],
                                             start=(jt == 0), stop=(jt == njt - 1))
                        # denominator: broadcast partition-sum, fast reciprocal
                        # (shares the av pool's two banks: av/dn alternate)
                        dps = av_ps.tile([P, CH], _F32, tag="av", name="dn")
                        nc.tensor.matmul(dps[:], lhsT=ones_mat[:], rhs=acc[:],
                                         start=True, stop=True)
                        bc_sb = bcpool.tile([P, CH], _F32, tag="bcs", name="bcs")
                        nc.vector.reciprocal_approx_fast(out=bc_sb[:], in_=dps[:])
                        ah = apool.tile([P, CH], _DT, tag=f"at{h}", name=f"at{h}")
                        nc.vector.tensor_tensor(
                            out=ah[:], in0=ops[:], in1=bc_sb[:],
                            op=mybir.AluOpType.mult)
                        at_t.append(ah)

                # W_o row-parallel: partial[i, 0:2048] from local heads only.
                # Collectives are issued only AFTER every po DMA of the chunk
                # (shared DMA-completion lanes: a collective in the middle of
                # the po stream makes later po waits wait on the whole RS).
                # collective split: one whole-chunk RS while the chain has
                # compute to hide behind (latency floor ~20us per collective),
                # four quarter-RS for the last chunk to shrink the exposed tail
                nsp = 4 if ic == NCH - 1 else 1
                spw = 4 // nsp
                with nc.named_scope(f"wo{ic}"):
                    bounces = [dpool.tile([4, spw, P, CH], _DT,
                                          tag=f"bounce{ic}_{sp}",
                                          name=f"bounce{sp}")
                               for sp in range(nsp)]
                    def emit_rs(sp):
                        # ReduceScatter(add): rank g of the batch group
                        # receives sum of partial[:, g*512:(g+1)*512]
                        rs_out = drpool.tile([spw * P, CH], _DT,
                                             tag=f"rso{spw}", name="rso")
                        nc.gpsimd.collective_compute(
                            "ReduceScatter", mybir.AluOpType.add,
                            replica_groups=[[0, 1, 2, 3], [4, 5, 6, 7]],
                            ins=[bounces[sp][:].opt()],
                            outs=[rs_out[:].opt()])
                        rs_outs.setdefault(ic, []).append((ic * 4 + sp * spw,
                                                           spw, rs_out))

                    for tl in range(4):
                        for o in range(4):
                            ps = mm_ps.tile([P, CH], _F32, tag="mm", name="wops")
                            for dt_ in range(NH):
                                nc.tensor.matmul(
                                    ps[:], lhsT=at_t[dt_][:, tl * P:(tl + 1) * P],
                                    rhs=wo_sb[:, dt_, o * CH:(o + 1) * CH],
                                    start=(dt_ == 0), stop=(dt_ == NH - 1))
                            # copy-outs alternate DVE/scalar: both queues stay
                            # shallow, so the mm_ps banks recycle faster (a
                            # full 16-copy backlog on either engine head-of-
                            # line blocks the next chunk's exp or presum work)
                            po = popool.tile([P, CH], _DT, tag="po", name="po")
                            if o % 2 == 0:
                                nc.vector.tensor_copy(out=po[:], in_=ps[:])
                            else:
                                nc.scalar.activation(
                                    out=po[:], in_=ps[:],
                                    func=mybir.ActivationFunctionType.Copy)
                            last_bounce = nc.sync.dma_start(
                                bounces[tl // spw][o, tl % spw], po[:])
                        if nsp == 4:
                            # last chunk: fire each quarter as its data lands
                            # (nothing queues behind it, so the mid-stream
                            # lane-coupling hazard doesn't apply here)
                            emit_rs(tl)
                    if nsp != 4:
                        for sp in range(nsp):
                            emit_rs(sp)
                # previous chunk's out-copies, pinned behind this chunk's last
                # bounce write: the tile scheduler otherwise hoists them right
                # behind their ReduceScatter, where the RS peer-wait head-of-
                # line blocks the issuing queue for the next chunk's work
                if ic > 0:
                    for tl0, spw_, rs_out in rs_outs.pop(ic - 1):
                        cp = nc.sync.dma_start(
                            out[tl0 * P:(tl0 + spw_) * P, :], rs_out[:])
                        tile.add_dep_helper(
                            cp.ins, last_bounce.ins, sync=False,
                            reason="out-copy after next chunk's bounces")
            for tl0, spw_, rs_out in rs_outs.pop(NCH - 1):
                cp = nc.sync.dma_start(out[tl0 * P:(tl0 + spw_) * P, :], rs_out[:])
                tile.add_dep_helper(cp.ins, last_bounce.ins, sync=False,
                                    reason="tail out-copy after last bounces")
        rpool.release()
        cpool.release()

    nc.compile()
    return nc


_CACHE = {}


def _get_nc(mode):
    if mode not in _CACHE:
        _CACHE[mode] = _build(mode)
    return _CACHE[mode]


def _tile_x(xt):
    """[D, S] -> [NCH, P, KT*CH] with [ic][p][t*CH+f] = xt[t*P+p][ic*CH+f]."""
    return np.ascontiguousarray(
        xt.reshape(KT, P, NCH, CH).transpose(2, 1, 0, 3).reshape(NCH, P, KT * CH))


def _tile_w(w):
    """[D, N] -> [P, KT*N] with [p][t*N+n] = w[t*P+p][n]."""
    n = w.shape[1]
    return np.ascontiguousarray(
        w.reshape(KT, P, n).transpose(1, 0, 2).reshape(P, KT * n))


def _tile_wo_rows(w):
    """[512, D] -> [P, NH*D] with [p][h*D+o] = w[h*128+p][o]."""
    return np.ascontiguousarray(
        w.reshape(NH, P, D).transpose(1, 0, 2).reshape(P, NH * D))


def _host_prep(q, k, v, mask, freq_cos, freq_sin, W_q, W_k, W_v, W_o):
    q = np.asarray(q, np.float32)
    k = np.asarray(k, np.float32)
    v = np.asarray(v, np.float32)
    W_q = np.asarray(W_q, np.float32)
    W_k = np.asarray(W_k, np.float32)
    W_v = np.asarray(W_v, np.float32)
    W_o = np.asarray(W_o, np.float32)
    cos = np.asarray(freq_cos, np.float32)
    sin = np.asarray(freq_sin, np.float32)
    mask = np.asarray(mask)

    tril = np.tril(np.ones((S, S), np.int32))
    if all(np.array_equal(mask[b], tril) for b in range(B)):
        mode = "causal"
    elif (mask == 1).all():
        mode = "full"
    else:
        mode = "addmask"

    # rope de-interleave permutation for head-dim pairing
    perm = np.concatenate([np.arange(0, HD, 2), np.arange(1, HD, 2)])
    cs = np.concatenate([cos.T, sin.T], axis=0).astype(BF16)   # [128, S]

    if mode == "causal":
        # transposed-scores diagonal block: sT[jj, ii] allowed iff jj <= ii
        jj = np.arange(P)[:, None]
        ii = np.arange(P)[None, :]
        cmask = np.where(jj <= ii, 0.0, -1e9).astype(np.float32).astype(BF16)

    in_maps = []
    for c in range(N_CORES):
        b, g = divmod(c, 4)
        wq_g = W_q[:, g * 512:(g + 1) * 512].copy()
        for l in range(NH):
            wq_g[:, l * HD:(l + 1) * HD] = wq_g[:, l * HD + perm]
        wk_g = W_k[:, g * HD:(g + 1) * HD][:, perm]
        wv_g = W_v[:, g * HD:(g + 1) * HD]
        wo_g = W_o[g * 512:(g + 1) * 512, :]
        m = {
            "xq": _tile_x(q[b].T.astype(BF16)),
            "xk": _tile_x(k[b].T.astype(BF16)),
            "xv": _tile_x(v[b].T.astype(BF16)),
            "wq": _tile_w(wq_g.astype(BF16)),
            "wk": _tile_w(wk_g.astype(BF16)),
            "wv": _tile_w(wv_g.astype(BF16)),
            "wo": _tile_wo_rows(wo_g.astype(BF16)),
            "cs": cs,
        }
        if mode == "causal":
            m["cmask"] = cmask
        elif mode == "addmask":
            # transposed orientation: amask[j, i]
            m["amask"] = np.ascontiguousarray(
                (mask[b].astype(np.float32).T - 1.0) * 1e9).astype(BF16)
        in_maps.append(m)
    return mode, in_maps


def kernel(q, k, v, mask, freq_cos, freq_sin, W_q, W_k, W_v, W_o,
           heads=16, group_size=4, _trace=False, _trace_kwargs=None):
    assert int(heads) == H and int(group_size) == G
    mode, in_maps = _host_prep(q, k, v, mask, freq_cos, freq_sin,
                               W_q, W_k, W_v, W_o)
    nc = _get_nc(mode)
    kw = {}
    if _trace:
        kw = dict(trace=True, **(_trace_kwargs or {}))
    res = run_bass_kernel_spmd(nc, in_maps, core_ids=list(range(N_CORES)), **kw)
    out = np.empty((B, S, D), np.float32)
    for c in range(N_CORES):
        b, g = divmod(c, 4)
        out[b, :, g * 512:(g + 1) * 512] = np.asarray(
            res.results[c]["out"]).astype(np.float32)
    if _trace:
        kernel._last_result = res
    return out
